# revision 21
# baseline (speedup 1.0000x reference)
"""LSTM (B=131072, T=10, INP=HID=64) + linear head, data-parallel on 8 TRN2 cores.

v7 layout (per core, B_loc=16384, 16 "units" of two 512-col groups A/B):
  - Feature-major: features on SBUF partitions, batch on the free dim. PSUM
    per unit-step: [128, 4, NB] banks (i, f, g, o), bank = [gate_A(0:64);
    gate_B(64:128)], so all elementwise ops run 128 lanes wide.
  - Both groups use rhs layout [h(0:64); x(64:128)] inside one persistent
    tile RAB[128, group, slot(4), NB] per unit, sharing a single weight copy.
    h_A lands aligned; h_B is written with a cross-partition output (legal:
    only tensor-op *inputs* must share a base partition). x is DMA'd two
    steps ahead into slot pairs, so DMA WAR waits are against long-retired
    readers and the SP queue never head-of-line blocks.
  - Bias: banks i, f seeded by K=1 matmuls on PE; bank g by a DVE copy from
    an SBUF image; the o-gate sigmoid is a separate ACT instruction carrying
    its bias as a per-partition vector (free). g weights/bias pre-doubled so
    tanh(g) = 2*sig(2g)-1. tanh(c) batched across unit pairs.
  - Two-phase software pipeline with a LAG-pair lead: phase 1 (matmuls,
    sigmoids, gate products, c update) runs ahead; phase 2 (tanh, h-muls,
    head) for pair q is emitted LAG pairs later, so the ACT queue never
    stalls on the cross-engine elementwise chain.
  - Elementwise: i*g and f*c products on Pool(GPSIMD); Gt fix, g-bank seed,
    c-add, h-muls, head staging on DVE.
"""

import numpy as np
import ml_dtypes

import concourse.bass as bass
import concourse.mybir as mybir
from concourse import bacc
import concourse.tile as tile

HID = 64
INP = 64
T = 10
B = 131072
NCORES = 8
B_LOC = B // NCORES   # 16384
NB = 512              # batch columns per group
NUNITS = B_LOC // (2 * NB)  # 16 units of (A, B) groups
NSLOT = 4             # rhs time slots (2-step DMA chunks, 2-step prefetch)
LAG = 2               # pairs of phase-1 lead over phase 2

BF = mybir.dt.bfloat16
F32 = mybir.dt.float32
AF = mybir.ActivationFunctionType
ALU = mybir.AluOpType

# psum gate-slice order matches torch block order: 0=i, 1=f, 2=g, 3=o
GATE_SCALE = [1.0, 1.0, 2.0, 1.0]


def emit_lstm(tc, aps):
    nc = tc.nc
    xab, Wd, BWd, BIgd, WOd, BOd, y = (
        aps["xab"], aps["Wd"], aps["BWd"], aps["BIgd"],
        aps["WOd"], aps["BOd"], aps["y"])

    with (
        tc.tile_pool(name="const", bufs=1) as cpool,
        tc.tile_pool(name="rhs", bufs=1) as rpool,
        tc.tile_pool(name="cstate", bufs=2) as spool,
        tc.tile_pool(name="gsif", bufs=3) as gifpool,
        tc.tile_pool(name="gsgo", bufs=8) as ggopool,
        tc.tile_pool(name="work", bufs=2) as wpool,
        tc.tile_pool(name="small", bufs=3) as qpool,
        tc.tile_pool(name="psum", bufs=2, space="PSUM") as ppool,
    ):
        W = cpool.tile([128, 4, 64], BF)    # [k(h;x), slice, m]
        nc.sync.dma_start(out=W, in_=Wd)
        BW = cpool.tile([1, 3, 128], BF)    # i, f, g seed rows
        nc.sync.dma_start(out=BW, in_=BWd)
        BIg = cpool.tile([128, NB], BF)     # o bank bias image
        nc.sync.dma_start(out=BIg, in_=BIgd)
        WO = cpool.tile([64, 1], BF)
        nc.sync.dma_start(out=WO, in_=WOd)
        BO = cpool.tile([1, 1], BF)
        nc.sync.dma_start(out=BO, in_=BOd)
        ones_sb = cpool.tile([1, NB], BF)
        nc.vector.memset(ones_sb, 1.0)

        # persistent rhs tiles: [h(0:64); x(64:128)] x group x slot
        R = [rpool.tile([128, 2, NSLOT, NB], BF, tag=f"r{u}", name=f"r_{u}")
             for u in range(NUNITS)]

        def x_dma(u, chunk):
            sl = (2 * chunk) % NSLOT
            nc.sync.dma_start(out=R[u][64:128, :, sl:sl + 2, :],
                              in_=xab[chunk, :, u])

        for u in range(NUNITS):
            x_dma(u, 0)

        CP = [(None, None)] * (NUNITS // 4)  # (new, old) c tiles per quad
        OO = {}                              # (t, u) -> o-gate sigmoid tile

        def phase1(t, u):
            p = u // 2
            sl = t % NSLOT
            ru = R[u]
            def gate_mms(ps, s0, s1):
                for s in (s0, s1):
                    lst = s == s1
                    if t == 0:
                        nc.tensor.matmul(ps[0:64, s - s0], W[64:128, s, :],
                                         ru[64:128, 0, 0, :], start=False,
                                         stop=False, skip_group_check=True)
                        nc.tensor.matmul(ps[64:128, s - s0], W[64:128, s, :],
                                         ru[64:128, 1, 0, :], start=False,
                                         stop=lst, skip_group_check=True)
                    else:
                        nc.tensor.matmul(ps[0:64, s - s0], W[:, s, :],
                                         ru[:, 0, sl, :], start=False,
                                         stop=False, skip_group_check=True)
                        nc.tensor.matmul(ps[64:128, s - s0], W[:, s, :],
                                         ru[:, 1, sl, :], start=False,
                                         stop=lst, skip_group_check=True)

            ps_if = ppool.tile([128, 2, NB], F32, tag="pif", name=f"pif_{t}_{u}")
            for s in range(2):
                nc.tensor.matmul(ps_if[:, s], BW[:, s, :], ones_sb,
                                 start=True, stop=False,
                                 skip_group_check=True)
            gate_mms(ps_if, 0, 1)
            GSif = gifpool.tile([128, 2, NB], BF, tag="GSif", name=f"gsif_{t}_{u}")
            nc.scalar.activation(GSif, ps_if, AF.Sigmoid)

            ps_go = ppool.tile([128, 2, NB], F32, tag="pgo", name=f"pgo_{t}_{u}")
            nc.tensor.matmul(ps_go[:, 0], BW[:, 2, :], ones_sb,
                             start=True, stop=False, skip_group_check=True)
            nc.vector.tensor_copy(out=ps_go[:, 1, :], in_=BIg)
            gate_mms(ps_go, 2, 3)
            GS = ggopool.tile([128, 2, NB], BF, tag="GSgo", name=f"gsgo_{t}_{u}")
            nc.scalar.activation(GS, ps_go, AF.Sigmoid)
            OO[(t, u)] = GS

            if u % 4 == 0:
                CPn = spool.tile([128, 4, NB], BF, tag=f"C{u // 4}",
                                 name=f"c_{t}_{u // 4}")
                CP[u // 4] = (CPn, CP[u // 4][0])
            CPnew, CPold = CP[u // 4]
            if t != 0:
                # f*c needs only the first sigmoid: start it early, off the
                # critical chain, on Pool
                ww = qpool.tile([128, NB], BF, tag="ww", name=f"ww_{t}_{u}")
                nc.gpsimd.tensor_mul(ww, GSif[:, 1], CPold[:, u % 4, :])
            Gt = qpool.tile([128, NB], BF, tag="Gt", name=f"gt_{t}_{u}")
            nc.vector.tensor_scalar(Gt, GS[:, 0], 2.0, -1.0,
                                    ALU.mult, ALU.add)
            uu = qpool.tile([128, NB], BF, tag="uu", name=f"uu_{t}_{u}")
            nc.vector.tensor_mul(uu, GSif[:, 0], Gt)
            if t == 0:
                nc.vector.tensor_copy(out=CPnew[:, u % 4, :], in_=uu)
            else:
                nc.vector.tensor_add(CPnew[:, u % 4, :], uu, ww)

        def phase2(t, q):
            last = t == T - 1
            TT = wpool.tile([128, 4, NB], BF, tag="TT", name=f"tt_{t}_{q}")
            nc.scalar.activation(TT, CP[q][0], AF.Tanh)
            for v in (4 * q, 4 * q + 1, 4 * q + 2, 4 * q + 3):
                O = OO.pop((t, v))[:, 1]
                if not last:
                    if t % 2 == 0 and t + 2 < T:
                        x_dma(v, (t + 2) // 2)
                    sln = (t + 1) % NSLOT
                    rv = R[v]
                    nc.vector.tensor_mul(rv[0:64, 0, sln, :],
                                         O[0:64, :], TT[0:64, v % 4])
                    nc.vector.tensor_mul(rv[0:64, 1, sln, :],
                                         O[64:128, :], TT[64:128, v % 4])
                else:
                    H = wpool.tile([64, 2, NB], BF, tag="TT", name=f"h_{v}")
                    nc.vector.tensor_mul(H[:, 0, :], O[0:64, :],
                                         TT[0:64, v % 4])
                    nc.vector.tensor_mul(H[:, 1, :], O[64:128, :],
                                         TT[64:128, v % 4])
                    ob = wpool.tile([1, 2, NB], BF, tag="TT", name=f"ob_{v}")
                    for g in range(2):
                        op = ppool.tile([1, NB], F32, tag="pif",
                                        name=f"op_{v}_{g}")
                        nc.tensor.matmul(op, BO, ones_sb,
                                         start=True, stop=False,
                                         skip_group_check=True)
                        nc.tensor.matmul(op, WO, H[:, g, :],
                                         start=False, stop=True,
                                         skip_group_check=True)
                        nc.vector.tensor_copy(out=ob[:, g, :], in_=op)
                    nc.sync.dma_start(out=y[v], in_=ob)

        pending = []
        for t in range(T):
            for u in range(NUNITS):
                phase1(t, u)
                if u % 4 == 3:
                    pending.append((t, u // 4))
                    if len(pending) > 1:
                        phase2(*pending.pop(0))
        while pending:
            phase2(*pending.pop(0))


def prep_weights(W_ih, W_hh, b_ih, b_hh, W_out, b_out):
    """Host-side packing (numpy). Returns DRAM arrays for the kernel."""
    bf16 = ml_dtypes.bfloat16
    W = np.zeros((128, 4, 64), np.float32)      # rhs layout [h; x]
    BIAS = np.zeros((4, 128), np.float32)
    b = (b_ih + b_hh).astype(np.float32)
    for s in range(4):
        blk_ih = W_ih[s * 64:(s + 1) * 64, :].astype(np.float32)
        blk_hh = W_hh[s * 64:(s + 1) * 64, :].astype(np.float32)
        scale = GATE_SCALE[s]
        W[0:64, s, :] = blk_hh.T * scale
        W[64:128, s, :] = blk_ih.T * scale
        bb = b[s * 64:(s + 1) * 64] * scale
        BIAS[s, 0:64] = bb
        BIAS[s, 64:128] = bb
    BW = BIAS[0:3][None, :, :]                   # [1, 3, 128] i, f, g seeds
    BIg = np.broadcast_to(BIAS[3][:, None], (128, NB))
    WO = W_out[0].astype(np.float32).reshape(64, 1)
    BO = np.full((1, 1), np.float32(b_out[0]))
    return {
        "Wd": W.astype(bf16),
        "BWd": np.ascontiguousarray(BW).astype(bf16),
        "BIgd": np.ascontiguousarray(BIg).astype(bf16),
        "WOd": WO.astype(bf16),
        "BOd": BO.astype(bf16),
    }


_BUILD_CACHE = {}


def build_nc():
    key = "nc_v7"
    if key in _BUILD_CACHE:
        return _BUILD_CACHE[key]
    nc = bacc.Bacc("TRN2", target_bir_lowering=False, debug=False)
    aps = {
        "xab": nc.dram_tensor("xab", [T // 2, INP, NUNITS, 2, 2, NB], BF,
                              kind="ExternalInput").ap(),
        "Wd": nc.dram_tensor("Wd", [128, 4, 64], BF,
                             kind="ExternalInput").ap(),
        "BWd": nc.dram_tensor("BWd", [1, 3, 128], BF, kind="ExternalInput").ap(),
        "BIgd": nc.dram_tensor("BIgd", [128, NB], BF,
                               kind="ExternalInput").ap(),
        "WOd": nc.dram_tensor("WOd", [64, 1], BF, kind="ExternalInput").ap(),
        "BOd": nc.dram_tensor("BOd", [1, 1], BF, kind="ExternalInput").ap(),
        "y": nc.dram_tensor("y", [NUNITS, 1, 2, NB], BF,
                            kind="ExternalOutput").ap(),
    }
    with tile.TileContext(nc) as tc:
        emit_lstm(tc, aps)
    nc.compile()
    _BUILD_CACHE[key] = nc
    return nc


def make_in_maps(x, W_ih, W_hh, b_ih, b_hh, W_out, b_out):
    bf16 = ml_dtypes.bfloat16
    wd = prep_weights(W_ih, W_hh, b_ih, b_hh, W_out, b_out)
    xt = np.ascontiguousarray(x.transpose(1, 2, 0))   # [T, I, B] f32
    in_maps = []
    for c in range(NCORES):
        sl = xt[:, :, c * B_LOC:(c + 1) * B_LOC]
        # [T, I, B_loc] -> [T/2(chunk), 2(step), I, NU, 2(grp), NB]
        blk = sl.reshape(T // 2, 2, INP, NUNITS, 2, NB)
        xab = np.ascontiguousarray(
            blk.transpose(0, 2, 3, 4, 1, 5)).astype(bf16)
        in_maps.append({"xab": xab, **wd})
    return in_maps


def kernel(x, W_ih, W_hh, b_ih, b_hh, W_out, b_out):
    from concourse.bass_utils import run_bass_kernel_spmd

    nc = build_nc()
    in_maps = make_in_maps(x, W_ih, W_hh, b_ih, b_hh, W_out, b_out)
    res = run_bass_kernel_spmd(nc, in_maps, core_ids=list(range(NCORES)))
    y = np.concatenate([res.results[c]["y"].astype(np.float32).reshape(B_LOC)
                        for c in range(NCORES)])
    return y.reshape(B, 1).astype(np.float32)


# revision 22
# speedup vs baseline: 1.1220x; 1.1220x over previous
"""LSTM (B=131072, T=10, INP=HID=64) + linear head, data-parallel on 8 TRN2 cores.

v7 layout (per core, B_loc=16384, 16 "units" of two 512-col groups A/B):
  - Feature-major: features on SBUF partitions, batch on the free dim. PSUM
    per unit-step: [128, 4, NB] banks (i, f, g, o), bank = [gate_A(0:64);
    gate_B(64:128)], so all elementwise ops run 128 lanes wide.
  - Both groups use rhs layout [h(0:64); x(64:128)] inside one persistent
    tile RAB[128, group, slot(4), NB] per unit, sharing a single weight copy.
    h_A lands aligned; h_B is written with a cross-partition output (legal:
    only tensor-op *inputs* must share a base partition). x is DMA'd two
    steps ahead into slot pairs, so DMA WAR waits are against long-retired
    readers and the SP queue never head-of-line blocks.
  - Bias: banks i, f seeded by K=1 matmuls on PE; bank g by a DVE copy from
    an SBUF image; the o-gate sigmoid is a separate ACT instruction carrying
    its bias as a per-partition vector (free). g weights/bias pre-doubled so
    tanh(g) = 2*sig(2g)-1. tanh(c) batched across unit pairs.
  - Two-phase software pipeline with a LAG-pair lead: phase 1 (matmuls,
    sigmoids, gate products, c update) runs ahead; phase 2 (tanh, h-muls,
    head) for pair q is emitted LAG pairs later, so the ACT queue never
    stalls on the cross-engine elementwise chain.
  - Elementwise: i*g and f*c products on Pool(GPSIMD); Gt fix, g-bank seed,
    c-add, h-muls, head staging on DVE.
"""

import numpy as np
import ml_dtypes

import concourse.bass as bass
import concourse.mybir as mybir
from concourse import bacc
import concourse.tile as tile

HID = 64
INP = 64
T = 10
B = 131072
NCORES = 8
B_LOC = B // NCORES   # 16384
NB = 512              # batch columns per group
NUNITS = B_LOC // (2 * NB)  # 16 units of (A, B) groups
NSLOT = 4             # rhs time slots (2-step DMA chunks, 2-step prefetch)
LAG = 2               # pairs of phase-1 lead over phase 2

BF = mybir.dt.bfloat16
F32 = mybir.dt.float32
AF = mybir.ActivationFunctionType
ALU = mybir.AluOpType

# psum gate-slice order matches torch block order: 0=i, 1=f, 2=g, 3=o
GATE_SCALE = [1.0, 1.0, 2.0, 1.0]


def emit_lstm(tc, aps):
    nc = tc.nc
    xab, Wd, BWd, BIgd, WOd, BOd, y = (
        aps["xab"], aps["Wd"], aps["BWd"], aps["BIgd"],
        aps["WOd"], aps["BOd"], aps["y"])

    with (
        tc.tile_pool(name="const", bufs=1) as cpool,
        tc.tile_pool(name="rhs", bufs=1) as rpool,
        tc.tile_pool(name="cstate", bufs=2) as spool,
        tc.tile_pool(name="gsif", bufs=4) as gifpool,
        tc.tile_pool(name="gsgo", bufs=6) as ggopool,
        tc.tile_pool(name="work", bufs=4) as wpool,
        tc.tile_pool(name="small", bufs=3) as qpool,
        tc.tile_pool(name="psum", bufs=2, space="PSUM") as ppool,
    ):
        W = cpool.tile([128, 4, 64], BF)    # [k(h;x), slice, m]
        nc.sync.dma_start(out=W, in_=Wd)
        BW = cpool.tile([1, 3, 128], BF)    # i, f, g seed rows
        nc.sync.dma_start(out=BW, in_=BWd)
        BIg = cpool.tile([128, NB], BF)     # o bank bias image
        nc.sync.dma_start(out=BIg, in_=BIgd)
        WO = cpool.tile([64, 1], BF)
        nc.sync.dma_start(out=WO, in_=WOd)
        BO = cpool.tile([1, 1], BF)
        nc.sync.dma_start(out=BO, in_=BOd)
        ones_sb = cpool.tile([1, NB], BF)
        nc.vector.memset(ones_sb, 1.0)

        # persistent rhs tiles: [h(0:64); x(64:128)] x group x slot
        R = [rpool.tile([128, 2, NSLOT, NB], BF, tag=f"r{u}", name=f"r_{u}")
             for u in range(NUNITS)]

        def x_dma(u, chunk):
            sl = (2 * chunk) % NSLOT
            nc.sync.dma_start(out=R[u][64:128, :, sl:sl + 2, :],
                              in_=xab[chunk, :, u])

        for u in range(NUNITS):
            x_dma(u, 0)

        CP = [(None, None)] * (NUNITS // 2)  # (new, old) c tiles per pair
        OO = {}                              # (t, u) -> o-gate sigmoid tile

        def phase1(t, u):
            p = u // 2
            sl = t % NSLOT
            ru = R[u]
            def gate_mms(ps, s0, s1):
                for s in (s0, s1):
                    lst = s == s1
                    if t == 0:
                        nc.tensor.matmul(ps[0:64, s - s0], W[64:128, s, :],
                                         ru[64:128, 0, 0, :], start=False,
                                         stop=False, skip_group_check=True)
                        nc.tensor.matmul(ps[64:128, s - s0], W[64:128, s, :],
                                         ru[64:128, 1, 0, :], start=False,
                                         stop=lst, skip_group_check=True)
                    else:
                        nc.tensor.matmul(ps[0:64, s - s0], W[:, s, :],
                                         ru[:, 0, sl, :], start=False,
                                         stop=False, skip_group_check=True)
                        nc.tensor.matmul(ps[64:128, s - s0], W[:, s, :],
                                         ru[:, 1, sl, :], start=False,
                                         stop=lst, skip_group_check=True)

            ps_if = ppool.tile([128, 2, NB], F32, tag="pif", name=f"pif_{t}_{u}")
            for s in range(2):
                nc.tensor.matmul(ps_if[:, s], BW[:, s, :], ones_sb,
                                 start=True, stop=False,
                                 skip_group_check=True)
            gate_mms(ps_if, 0, 1)
            GSif = gifpool.tile([128, 2, NB], BF, tag="GSif", name=f"gsif_{t}_{u}")
            nc.scalar.activation(GSif, ps_if, AF.Sigmoid)

            ps_go = ppool.tile([128, 2, NB], F32, tag="pgo", name=f"pgo_{t}_{u}")
            nc.tensor.matmul(ps_go[:, 0], BW[:, 2, :], ones_sb,
                             start=True, stop=False, skip_group_check=True)
            nc.vector.tensor_copy(out=ps_go[:, 1, :], in_=BIg)
            gate_mms(ps_go, 2, 3)
            GS = ggopool.tile([128, 2, NB], BF, tag="GSgo", name=f"gsgo_{t}_{u}")
            nc.scalar.activation(GS, ps_go, AF.Sigmoid)
            OO[(t, u)] = GS

            if u % 2 == 0:
                CPn = spool.tile([128, 2, NB], BF, tag=f"C{p}",
                                 name=f"c_{t}_{p}")
                CP[p] = (CPn, CP[p][0])
            CPnew, CPold = CP[p]
            if t != 0:
                # f*c needs only the first sigmoid: start it early, off the
                # critical chain, on Pool
                ww = qpool.tile([128, NB], BF, tag="ww", name=f"ww_{t}_{u}")
                nc.gpsimd.tensor_mul(ww, GSif[:, 1], CPold[:, u % 2, :])
            Gt = qpool.tile([128, NB], BF, tag="Gt", name=f"gt_{t}_{u}")
            nc.vector.tensor_scalar(Gt, GS[:, 0], 2.0, -1.0,
                                    ALU.mult, ALU.add)
            uu = qpool.tile([128, NB], BF, tag="uu", name=f"uu_{t}_{u}")
            nc.vector.tensor_mul(uu, GSif[:, 0], Gt)
            if t == 0:
                nc.vector.tensor_copy(out=CPnew[:, u % 2, :], in_=uu)
            else:
                nc.vector.tensor_add(CPnew[:, u % 2, :], uu, ww)

        def phase2(t, q):
            last = t == T - 1
            TT = wpool.tile([128, 2, NB], BF, tag="TT", name=f"tt_{t}_{q}")
            nc.scalar.activation(TT, CP[q][0], AF.Tanh)
            for v in (2 * q, 2 * q + 1):
                O = OO.pop((t, v))[:, 1]
                if not last:
                    if t % 2 == 0 and t + 2 < T:
                        x_dma(v, (t + 2) // 2)
                    sln = (t + 1) % NSLOT
                    rv = R[v]
                    nc.vector.tensor_mul(rv[0:64, 0, sln, :],
                                         O[0:64, :], TT[0:64, v % 2])
                    nc.vector.tensor_mul(rv[0:64, 1, sln, :],
                                         O[64:128, :], TT[64:128, v % 2])
                else:
                    H = wpool.tile([64, 2, NB], BF, tag="TT", name=f"h_{v}")
                    nc.vector.tensor_mul(H[:, 0, :], O[0:64, :],
                                         TT[0:64, v % 2])
                    nc.vector.tensor_mul(H[:, 1, :], O[64:128, :],
                                         TT[64:128, v % 2])
                    ob = wpool.tile([1, 2, NB], BF, tag="TT", name=f"ob_{v}")
                    for g in range(2):
                        op = ppool.tile([1, NB], F32, tag="pif",
                                        name=f"op_{v}_{g}")
                        nc.tensor.matmul(op, BO, ones_sb,
                                         start=True, stop=False,
                                         skip_group_check=True)
                        nc.tensor.matmul(op, WO, H[:, g, :],
                                         start=False, stop=True,
                                         skip_group_check=True)
                        nc.vector.tensor_copy(out=ob[:, g, :], in_=op)
                    nc.sync.dma_start(out=y[v], in_=ob)

        pending = []
        for t in range(T):
            for u in range(NUNITS):
                phase1(t, u)
                if u % 2 == 1:
                    pending.append((t, u // 2))
                    if len(pending) > LAG:
                        phase2(*pending.pop(0))
        while pending:
            phase2(*pending.pop(0))


def prep_weights(W_ih, W_hh, b_ih, b_hh, W_out, b_out):
    """Host-side packing (numpy). Returns DRAM arrays for the kernel."""
    bf16 = ml_dtypes.bfloat16
    W = np.zeros((128, 4, 64), np.float32)      # rhs layout [h; x]
    BIAS = np.zeros((4, 128), np.float32)
    b = (b_ih + b_hh).astype(np.float32)
    for s in range(4):
        blk_ih = W_ih[s * 64:(s + 1) * 64, :].astype(np.float32)
        blk_hh = W_hh[s * 64:(s + 1) * 64, :].astype(np.float32)
        scale = GATE_SCALE[s]
        W[0:64, s, :] = blk_hh.T * scale
        W[64:128, s, :] = blk_ih.T * scale
        bb = b[s * 64:(s + 1) * 64] * scale
        BIAS[s, 0:64] = bb
        BIAS[s, 64:128] = bb
    BW = BIAS[0:3][None, :, :]                   # [1, 3, 128] i, f, g seeds
    BIg = np.broadcast_to(BIAS[3][:, None], (128, NB))
    WO = W_out[0].astype(np.float32).reshape(64, 1)
    BO = np.full((1, 1), np.float32(b_out[0]))
    return {
        "Wd": W.astype(bf16),
        "BWd": np.ascontiguousarray(BW).astype(bf16),
        "BIgd": np.ascontiguousarray(BIg).astype(bf16),
        "WOd": WO.astype(bf16),
        "BOd": BO.astype(bf16),
    }


_BUILD_CACHE = {}


def build_nc():
    key = "nc_v7"
    if key in _BUILD_CACHE:
        return _BUILD_CACHE[key]
    nc = bacc.Bacc("TRN2", target_bir_lowering=False, debug=False)
    aps = {
        "xab": nc.dram_tensor("xab", [T // 2, INP, NUNITS, 2, 2, NB], BF,
                              kind="ExternalInput").ap(),
        "Wd": nc.dram_tensor("Wd", [128, 4, 64], BF,
                             kind="ExternalInput").ap(),
        "BWd": nc.dram_tensor("BWd", [1, 3, 128], BF, kind="ExternalInput").ap(),
        "BIgd": nc.dram_tensor("BIgd", [128, NB], BF,
                               kind="ExternalInput").ap(),
        "WOd": nc.dram_tensor("WOd", [64, 1], BF, kind="ExternalInput").ap(),
        "BOd": nc.dram_tensor("BOd", [1, 1], BF, kind="ExternalInput").ap(),
        "y": nc.dram_tensor("y", [NUNITS, 1, 2, NB], BF,
                            kind="ExternalOutput").ap(),
    }
    with tile.TileContext(nc) as tc:
        emit_lstm(tc, aps)
    nc.compile()
    _BUILD_CACHE[key] = nc
    return nc


def make_in_maps(x, W_ih, W_hh, b_ih, b_hh, W_out, b_out):
    bf16 = ml_dtypes.bfloat16
    wd = prep_weights(W_ih, W_hh, b_ih, b_hh, W_out, b_out)
    xt = np.ascontiguousarray(x.transpose(1, 2, 0))   # [T, I, B] f32
    in_maps = []
    for c in range(NCORES):
        sl = xt[:, :, c * B_LOC:(c + 1) * B_LOC]
        # [T, I, B_loc] -> [T/2(chunk), 2(step), I, NU, 2(grp), NB]
        blk = sl.reshape(T // 2, 2, INP, NUNITS, 2, NB)
        xab = np.ascontiguousarray(
            blk.transpose(0, 2, 3, 4, 1, 5)).astype(bf16)
        in_maps.append({"xab": xab, **wd})
    return in_maps


def kernel(x, W_ih, W_hh, b_ih, b_hh, W_out, b_out):
    from concourse.bass_utils import run_bass_kernel_spmd

    nc = build_nc()
    in_maps = make_in_maps(x, W_ih, W_hh, b_ih, b_hh, W_out, b_out)
    res = run_bass_kernel_spmd(nc, in_maps, core_ids=list(range(NCORES)))
    y = np.concatenate([res.results[c]["y"].astype(np.float32).reshape(B_LOC)
                        for c in range(NCORES)])
    return y.reshape(B, 1).astype(np.float32)
